# revision 54
# baseline (speedup 1.0000x reference)
"""Trainium2 Bass kernel for nn_Beta_LR_41308995453190.

Network (per (b, o) pair):
  - 13 segment means over the L axis of hidden[b, o] (ragged boundaries
    from idx[b]): 10 context segments, question, option, whole-context.
  - beta-param projection e = 1 + relu(x @ Wp + bp), split a/b.
  - three attention pools (intersection over segments, renew over
    (segment, intersection) pairs, union over inverted renewed params).
  - classify head: concat 8 beta embeddings -> relu(@Wl0 + bl0) -> @Wl + bl.

Sharding: data-parallel over the batch dim B=8 (one batch per NeuronCore),
weights replicated.

Design (v2 — rebuilt around the trace of the v1 kernel):
  - hidden travels in fp8 e3m4 (4.2 MB/core instead of 8.4 bf16; measured
    end-to-end rel-err 2.9e-3 vs the 2e-2 gate). All weights bf16.
  - Segment sums are 0/1-mask matmuls (mask stationary, hidden streaming).
    The two E-halves run CONCURRENTLY in separate PE column groups
    (tile_position col 0 / 32, derived from the PSUM slice base partition).
  - The beta-network layers run in CLASSIC orientation: weight chunks
    [128, 128] stationary, feature-major activations [128, cols] streaming.
    Layer outputs land feature-major in PSUM, so the bias/relu DVE op is
    128-partition-parallel and NO transposes are needed between layers
    (v1 spent ~10us of PE time on 40 transposes + PSUM round trips).
  - Softmaxes skip the max-subtraction (logits are ~N(0, 0.25)); the
    intersection's exp/weighted sums are reused by the renew stage.
  - Classify head: catF chunks stationary [128, 4], wl0 streams 512 wide,
    accumulated in 4 PE column groups concurrently; bl0 is folded in as a
    33rd contraction chunk (one-hot stationary, bl0 in wl0 row 0). The 24
    chunks that only need the projection run inside the softmax bubbles;
    epilogue relu*Wl+reduce is one fused scalar_tensor_tensor op.
  - DMA: hidden kicks on the Sync HWDGE queue, weights on the Scalar
    queue (two engines issue descriptors concurrently; each descriptor
    fans out to one of 16 HW DMA engines). wl0 (4.2 MB) is ordered last
    — the head only needs it ~25us in.
"""

import numpy as np
import ml_dtypes

try:
    import concourse.bass as bass
except ImportError:
    import sys

    sys.path.insert(0, "/opt/trn_rl_repo")
    import concourse.bass as bass

import concourse.tile as tile
from concourse import mybir
from concourse.bass_utils import run_bass_kernel_spmd
from concourse.masks import make_identity

F32 = mybir.dt.float32
BF16 = mybir.dt.bfloat16
FP8 = mybir.dt.float8e3  # e3m4
NPBF16 = ml_dtypes.bfloat16
NPFP8 = ml_dtypes.float8_e3m4
AX = mybir.AxisListType.X
OP = mybir.AluOpType
AF = mybir.ActivationFunctionType

B, O, L, E = 8, 4, 1024, 1024
BETA = 512
NSEG = 12
NK = 13  # 10 ctx + q + o + allc
NC10 = 10  # pooled ctx segments
P = 128
T = L // P  # 8 L-tiles per option
NCOL = O * NK  # 52

# wl0 host chunk order: natural (the head runs at the very end, after wl0
# has fully arrived; issuing it earlier would stall the in-order PE queue).
WL0_ORDER = list(range(33))
WL0_POS = {kc: i for i, kc in enumerate(WL0_ORDER)}


def _patch_minimal_drain():
    """One-shot NEFF: skip the semaphore-clear + second all-engine barrier of
    the TileContext epilogue (they only matter when the program loops).
    Worth ~6us of measured exec time (the ~150 per-engine semaphore clears
    serialize at 20-50ns each)."""
    from concourse.vector_clock import ScopedClock

    def _drain_and_barrier(self, tick_clock, wait_clock):
        drain_inst = self.nc.sync.drain()
        wait_clock.add_sem_waits(
            drain_inst.ins, ScopedClock({None: tick_clock.global_clock})
        )
        assert self.sems is not None
        popped = self.nc._tile_sem_poison_stack.pop()
        assert popped is self._sem_poison
    tile.TileContext._drain_and_barrier = _drain_and_barrier


_patch_minimal_drain()


def _split_excess_waits(nc, max_waits=1):
    """This neuronxcc walrus build rejects more than one sem wait per TPB
    instruction ("Too many sync wait commands"). Hoist excess waits onto
    drain instructions inserted immediately before the offending instruction
    on the same engine."""
    scratch_bb = nc.cur_bb.bb
    for f in nc.m.functions:
        for bb in f.blocks:
            new_list = []
            for ins in bb.instructions:
                si = ins.sync_info
                waits = list(si.on_wait) if si and si.on_wait else []
                if len(waits) > max_waits:
                    for w in waits[: len(waits) - max_waits]:
                        carrier = nc.engines[ins.engine].nop(nofuse=True).ins
                        scratch_bb.instructions.remove(carrier)
                        carrier.sync_info = mybir.SyncInfo(
                            on_wait=[w], on_update=[]
                        )
                        new_list.append(carrier)
                    si.on_wait = waits[len(waits) - max_waits :]
                new_list.append(ins)
            bb.instructions[:] = new_list


def _build_nc(debug=False):
    nc = bass.Bass("TRN2", target_bir_lowering=False)

    hid_d = nc.dram_tensor("hidden", [O, P, T, E], FP8, kind="ExternalInput")
    mask_d = nc.dram_tensor("maskt", [P, T, NK], FP8, kind="ExternalInput")
    cnt_d = nc.dram_tensor("cntinv", [NK, 1], F32, kind="ExternalInput")
    wp_d = nc.dram_tensor("wp", [P, 8, 1024], FP8, kind="ExternalInput")
    wa0_d = nc.dram_tensor("wa0", [P, 8, 512], FP8, kind="ExternalInput")
    wa_d = nc.dram_tensor("wa", [P, 4, 512], FP8, kind="ExternalInput")
    wl0_d = nc.dram_tensor("wl0", [P, 33, 512], BF16, kind="ExternalInput")
    biasx_d = nc.dram_tensor("biasx", [1, 16, P], BF16, kind="ExternalInput")
    wlr_d = nc.dram_tensor("wlrep", [O, 513], F32, kind="ExternalInput")
    out_d = nc.dram_tensor("out", [O, 1], F32, kind="ExternalOutput")

    with tile.TileContext(nc) as tc:
        with (
            tc.tile_pool(name="const", bufs=1) as const,
            tc.tile_pool(name="hidp2", bufs=2) as hidp2,
            tc.tile_pool(name="act", bufs=1) as act,
            tc.tile_pool(name="tmp", bufs=2) as tmp,
            tc.tile_pool(name="pseg", bufs=2, space="PSUM") as pseg,
            tc.tile_pool(name="pt", bufs=2, space="PSUM") as pt,
            tc.tile_pool(name="pmm", bufs=2, space="PSUM") as pmm,
            tc.tile_pool(name="phead", bufs=1, space="PSUM") as phead,
        ):
            # ---- DMA kicks. Two HWDGE queues (Sync, Scalar) issue
            # descriptors concurrently; each descriptor fans out to one of
            # 16 HW DMA engines (~24.5 GB/s per engine, ~358 GB/s aggregate,
            # FIFO per engine). Hidden options are interleaved across both
            # queues so all 16 hidden chunks land on distinct engines first;
            # weights follow (wp/wa0 split by output block so the consuming
            # layers pipeline); wl0 is last (head needs it at ~40us).
            mask_sb = const.tile([P, T, NK], FP8)
            nc.sync.dma_start(out=mask_sb, in_=mask_d[:])
            # touch the activation table now so ACT_TABLE_LOAD (~1.3us)
            # overlaps the initial DMA instead of stalling the first use
            actwarm = const.tile([1, 1], F32)
            nc.vector.memset(actwarm, 1.0)
            nc.scalar.activation(out=actwarm, in_=actwarm, func=AF.Exp)
            cnt_sb = const.tile([NK, 1], F32)
            nc.scalar.dma_start(out=cnt_sb, in_=cnt_d[:])
            wlr_sb = const.tile([O, 513], F32)
            nc.scalar.dma_start(out=wlr_sb, in_=wlr_d[:])
            # bias-as-matmul: every layer chain gets one extra contraction
            # chunk (ones-row moving x bias-row stationary, both bf16) so
            # the PSUM already carries the bias and the downstream relu/exp
            # is a single wide op with no per-m-chunk bias columns.
            biasx_sb = const.tile([P, 16, P], BF16)
            nc.vector.memset(biasx_sb, 0.0)
            nc.scalar.dma_start(out=biasx_sb[0:1, :, :], in_=biasx_d[:])
            ones52 = const.tile([P, O, NK], BF16)
            nc.vector.memset(ones52, 0.0)
            nc.vector.memset(ones52[0:1, :, :], 1.0)

            # Few big descriptors in strict need-order: each descriptor
            # sustains ~170 GB/s on its HW engine, so ~2 in flight saturate
            # the ~358 GB/s aggregate. One option = 2 descriptors (one per
            # HWDGE queue); weights follow; wl0 last (head needs it ~45us).
            # A queue round-robins across ALL its pending descriptors, so
            # queue position gives no priority and concurrent transfers
            # steal bandwidth from hidden. Build a strict transfer pipeline
            # o0 -> o1 -> o2 -> o3 -> wp -> wa0/wa -> wl0 by gating each
            # kick on the previous tensor's LAST bytes via 1-element dummy
            # copies (WAW dep -> sem wait on the kick). Sequential options
            # also keep the PE continuously fed (HAM stays warm).
            hid_r = hid_d
            htiles = [
                hidp2.tile([P, T, E], FP8, tag="htile", name=f"htile{o}")
                for o in range(O)
            ]

            def gate(dst_tile, srcs):
                for j, s in enumerate(srcs):
                    nc.gpsimd.tensor_copy(
                        out=dst_tile[0:1, 0, j : j + 1], in_=s
                    )

            def hid_last(o):
                # any element works: the DMA semaphore bumps once per
                # descriptor, so the dep resolves at descriptor completion
                return [htiles[o][0:1, 3, 0:1], htiles[o][0:1, 7, 0:1]]

            for o in range(O):
                nc.sync.dma_start(
                    out=htiles[o][:, 0:4, :], in_=hid_r[o][:, 0:4, :]
                )
                nc.scalar.dma_start(
                    out=htiles[o][:, 4:8, :], in_=hid_r[o][:, 4:8, :]
                )
            wp_sb = const.tile([P, 8, 1024], FP8)
            gate(wp_sb, hid_last(1))
            nc.sync.dma_start(out=wp_sb[:, 0:4, :], in_=wp_d[:, 0:4, :])
            nc.scalar.dma_start(out=wp_sb[:, 4:8, :], in_=wp_d[:, 4:8, :])
            wp_last = [wp_sb[0:1, 3, 0:1], wp_sb[0:1, 7, 0:1]]

            wa0_sb = const.tile([P, 8, 512], FP8)
            gate(wa0_sb, hid_last(2))
            wa_sb = const.tile([P, 4, 512], FP8)
            gate(wa_sb, hid_last(2))
            nc.sync.dma_start(out=wa0_sb, in_=wa0_d[:])
            nc.scalar.dma_start(out=wa_sb, in_=wa_d[:])
            wl0_sb = const.tile([P, 33, 512], BF16)
            gate(wl0_sb, hid_last(3))
            for j, sl in enumerate(
                (slice(0, 8), slice(8, 16), slice(16, 25), slice(25, 33))
            ):
                eng = nc.sync if j % 2 == 0 else nc.scalar
                eng.dma_start(out=wl0_sb[:, sl, :], in_=wl0_d[:, sl, :])

            ident = const.tile([P, P], BF16)
            make_identity(nc, ident)

            # ---- phase A: segment sums. ps[k, e] = sum over rows of seg k.
            # The two E-halves run concurrently in PE col groups 0 and 1
            # (group from the PSUM slice base partition: 0 -> cols 0-31,
    # 32 -> cols 32-63).
            x_all = act.tile([P, E], BF16, tag="x_all")
            for o in range(O):
                htile = htiles[o]
                ps = pseg.tile([P, 512], F32, tag="ps_seg")
                for t in range(T):
                    nc.tensor.matmul(
                        out=ps[0:NK, :],
                        lhsT=mask_sb[:, t, :],
                        rhs=htile[:, t, 0:512],
                        start=(t == 0),
                        stop=(t == T - 1),
                        skip_group_check=True,
                    )
                    nc.tensor.matmul(
                        out=ps[32 : 32 + NK, :],
                        lhsT=mask_sb[:, t, :],
                        rhs=htile[:, t, 512:1024],
                        start=(t == 0),
                        stop=(t == T - 1),
                        skip_group_check=True,
                    )
                nc.vector.tensor_scalar_mul(
                    out=x_all[o * 32 : o * 32 + NK, 0:512],
                    in0=ps[0:NK, :],
                    scalar1=cnt_sb[:, :],
                )
                nc.vector.tensor_scalar_mul(
                    out=x_all[o * 32 : o * 32 + NK, 512:1024],
                    in0=ps[32 : 32 + NK, :],
                    scalar1=cnt_sb[:, :],
                )

            # ---- xT: transpose to feature-major [128, 8, O, NK] bf16
            xTb = act.tile([P, 8, O, 32], BF16)
            for c in range(8):
                ptile = pt.tile([P, P], BF16, tag="pt")
                nc.tensor.transpose(
                    out=ptile,
                    in_=x_all[:, c * P : (c + 1) * P],
                    identity=ident[:, :],
                )
                nc.scalar.copy(
                    out=xTb[:, c, :, 0:NK],
                    in_=ptile.rearrange("p (o k) -> p o k", k=32)[:, :, 0:NK],
                )

            # ---- e = max(x @ Wp + (bp+1), 1), classic orientation:
            # Wp [128,128] chunks stationary, xTb streams 52 cols. k-outer
            # so the matmuls start as each wp k-chunk lands from DMA.
            # NOTE: chains must run one-at-a-time (m-outer): a matmul with
            # start=True clears its full partition range in the target bank,
            # so interleaving same-partition accumulation chains loses the
            # earlier chains' first terms (verified on HW).
            pe_ps = pmm.tile([P, 8, O, NK], F32, tag="pml")
            for m in range(8):
                nc.tensor.matmul(
                    out=pe_ps[:, m, :, :], lhsT=biasx_sb[:, m, :],
                    rhs=ones52, start=True, stop=False,
                )
                for k in range(8):
                    nc.tensor.matmul(
                        out=pe_ps[:, m, :, :],
                        lhsT=wp_sb[:, k, m * P : (m + 1) * P],
                        rhs=xTb[:, k, :, 0:NK],
                        start=False,
                        stop=(k == 7),
                    )
            # erf = relu(z + bp) on the scalar engine (pipelines behind the
            # m-chunk psums); e = erf + 1 is ONE wide vector op. Pool sums
            # use erf via fused (erf + 1) * w scalar_tensor_tensor ops.
            erf = act.tile([P, 8, O, NK], F32)
            nc.scalar.activation(out=erf, in_=pe_ps, func=AF.Relu)
            eTb = act.tile([P, 8, O, NK], BF16)
            nc.vector.tensor_scalar_add(out=eTb, in0=erf, scalar1=1.0)

            # catF chunks 8..31 (a_ac,b_ac,a_o,b_o,a_q,b_q) only need e.
            catFb = act.tile([P, 33, O], BF16)
            for j, (half, k) in enumerate(
                ((0, 12), (1, 12), (0, 11), (1, 11), (0, 10), (1, 10))
            ):
                nc.gpsimd.tensor_copy(
                    out=catFb[:, 8 + j * 4 : 12 + j * 4, :],
                    in_=eTb[:, half * 4 : half * 4 + 4, :, k],
                )
            # bl0 folding chunk: one-hot stationary row (partition 0 = 1).
            nc.gpsimd.memset(catFb[:, 32, :], 0.0)
            nc.gpsimd.memset(catFb[0:1, 32, :], 1.0)

            def layer(name, w_sb, nk_chunks, rhs_fn, nm, out_free, brow, ones):
                psl = pmm.tile(
                    [P, nm] + out_free, F32, tag="pml", name=f"ps_{name}"
                )
                for m in range(nm):
                    nc.tensor.matmul(
                        out=psl[:, m], lhsT=biasx_sb[:, brow + m, :],
                        rhs=ones, start=True, stop=False,
                    )
                    for k in range(nk_chunks):
                        nc.tensor.matmul(
                            out=psl[:, m],
                            lhsT=w_sb[:, k, m * P : (m + 1) * P],
                            rhs=rhs_fn(k),
                            start=False,
                            stop=(k == nk_chunks - 1),
                        )
                return psl

            # ---- pool 1 (intersection) over the 10 ctx segments.
            # Bias+relu and bias+exp run as single scalar-engine activation
            # ops straight out of PSUM (out = func(in*scale + bias)).
            ones40 = ones52[:, :, 0:NC10]
            ph1 = layer("h1", wa0_sb, 8, lambda k: eTb[:, k, :, 0:NC10], 4,
                        [O, NC10], 8, ones40)
            h1b = act.tile([P, 4, O, NC10], BF16)
            nc.vector.tensor_scalar(
                out=h1b, in0=ph1, scalar1=1.0 / 128.0, scalar2=0.0,
                op0=OP.mult, op1=OP.max,
            )
            pl1 = layer("l1", wa_sb, 4, lambda k: h1b[:, k], 4, [O, NC10],
                        12, ones40)

            # softmax over segments, no max-subtraction (|l1| < ~1.5).
            # Shared factors (w1, r1, ...) are duplicated across the a/b
            # chunk halves by gpsimd copies so each chain step is ONE wide
            # vector op instead of two serialized halves.
            w1x2 = act.tile([P, 8, O, NC10], F32)
            w1 = w1x2[:, 0:4]
            nc.scalar.activation(
                out=w1, in_=pl1, func=AF.Exp, scale=1.0 / 128.0
            )
            nc.gpsimd.tensor_copy(out=w1x2[:, 4:8], in_=w1)
            s1 = act.tile([P, 4, O], F32)
            nc.vector.reduce_sum(s1, w1, axis=AX)
            r1x2 = act.tile([P, 8, O], F32)
            nc.vector.reciprocal(out=r1x2[:, 0:4, :], in_=s1)
            nc.gpsimd.tensor_copy(out=r1x2[:, 4:8, :], in_=r1x2[:, 0:4, :])
            wab_t = act.tile([P, 8, O, NC10], F32)
            nc.vector.scalar_tensor_tensor(
                out=wab_t, in0=erf[:, :, :, 0:NC10], scalar=1.0, in1=w1x2,
                op0=OP.add, op1=OP.mult,
            )
            sumab = tmp.tile([P, 8, O], F32, tag="sumab")
            nc.vector.reduce_sum(sumab, wab_t, axis=AX)
            cat2 = act.tile([P, 8, O], F32)
            nc.vector.tensor_tensor(out=cat2, in0=sumab, in1=r1x2, op=OP.mult)
            cat2b = act.tile([P, 8, O], BF16)
            nc.vector.tensor_copy(out=cat2b, in_=cat2)

            # ---- renew: h2/l2 on the intersection [O] columns
            ones4 = ones52[:, :, 0:1]
            ph2 = layer("h2", wa0_sb, 8, lambda k: cat2b[:, k, :], 4, [O],
                        8, ones4)
            h2b = act.tile([P, 4, O], BF16)
            nc.vector.tensor_scalar(
                out=h2b, in0=ph2, scalar1=1.0 / 128.0, scalar2=0.0,
                op0=OP.mult, op1=OP.max,
            )
            pl2 = layer("l2", wa_sb, 4, lambda k: h2b[:, k], 4, [O], 12, ones4)

            # pair softmax([l1, l2]) -> na/nb, store reciprocals.
            # e1 = exp(l1) = w1 (reused), e2 = exp(l2).
            def warm_fill(n):
                # independent matmuls into a scratch bank: keep the PE HAM
                # at K=8/8 through the softmax bubbles (idle >3.4us would
                # re-throttle the clock to 1.2 GHz and slow the next burst)
                pscr = pt.tile([P, 512], F32, tag="pt", name="pscr")
                for i in range(n):
                    nc.tensor.matmul(
                        out=pscr[0:O, :], lhsT=catFb[:, 8, :],
                        rhs=wl0_sb[:, 0, :], start=True, stop=True,
                        skip_group_check=True,
                    )

            e2x2 = act.tile([P, 8, O], F32)
            e2 = e2x2[:, 0:4, :]
            nc.scalar.activation(out=e2, in_=pl2, func=AF.Exp, scale=1.0 / 128.0)
            nc.gpsimd.tensor_copy(out=e2x2[:, 4:8, :], in_=e2)
            s12 = tmp.tile([P, 4, O, NC10], F32, tag="s12")
            nc.vector.tensor_tensor(
                out=s12, in0=w1, in1=e2.broadcast_to([P, 4, O, NC10]), op=OP.add
            )
            s12x2 = act.tile([P, 8, O, NC10], F32)
            nc.gpsimd.tensor_copy(out=s12x2[:, 0:4], in_=s12)
            nc.gpsimd.tensor_copy(out=s12x2[:, 4:8], in_=s12)
            t2ab = tmp.tile([P, 8, O], F32, tag="t2ab")
            nc.vector.tensor_tensor(out=t2ab, in0=e2x2, in1=cat2, op=OP.mult)
            t3ab = tmp.tile([P, 8, O, NC10], F32, tag="t3ab")
            nc.vector.tensor_tensor(
                out=t3ab, in0=wab_t,
                in1=t2ab.broadcast_to([P, 8, O, NC10]), op=OP.add,
            )
            # 1/na = s12 * (1/(w1*a + e2*ia)): one wide reciprocal + mult
            # (DVE/Pool reject op=divide; two-recip version costs 1.2us more)
            rt3 = tmp.tile([P, 8, O, NC10], F32, tag="rt3")
            nc.vector.reciprocal(out=rt3, in_=t3ab)
            rab = act.tile([P, 8, O, NC10], F32)
            nc.vector.tensor_tensor(out=rab, in0=s12x2, in1=rt3, op=OP.mult)
            rabb = act.tile([P, 8, O, NC10], BF16)
            nc.vector.tensor_copy(out=rabb, in_=rab)

            # ---- union pool over segments of [1/na; 1/nb]
            ph3 = layer("h3", wa0_sb, 8, lambda k: rabb[:, k], 4,
                        [O, NC10], 8, ones40)
            h3b = act.tile([P, 4, O, NC10], BF16)
            nc.vector.tensor_scalar(
                out=h3b, in0=ph3, scalar1=1.0 / 128.0, scalar2=0.0,
                op0=OP.mult, op1=OP.max,
            )
            pl3 = layer("l3", wa_sb, 4, lambda k: h3b[:, k], 4, [O, NC10],
                        12, ones40)

            w3x2 = act.tile([P, 8, O, NC10], F32)
            w3 = w3x2[:, 0:4]
            nc.scalar.activation(out=w3, in_=pl3, func=AF.Exp, scale=1.0 / 128.0)
            nc.gpsimd.tensor_copy(out=w3x2[:, 4:8], in_=w3)
            s3x2 = act.tile([P, 8, O], F32)
            nc.vector.reduce_sum(s3x2[:, 0:4, :], w3, axis=AX)
            nc.gpsimd.tensor_copy(out=s3x2[:, 4:8, :], in_=s3x2[:, 0:4, :])
            # ua = s3 / sum(w3 * ra) ; ub likewise, one wide op per step
            tuab = tmp.tile([P, 8, O, NC10], F32, tag="tuab")
            nc.vector.tensor_tensor(out=tuab, in0=w3x2, in1=rab, op=OP.mult)
            suab = tmp.tile([P, 8, O], F32, tag="suab")
            nc.vector.reduce_sum(suab, tuab, axis=AX)
            invab = tmp.tile([P, 8, O], F32, tag="invab")
            nc.vector.reciprocal(out=invab, in_=suab)
            nc.vector.tensor_tensor(
                out=catFb[:, 0:8, :], in0=s3x2, in1=invab, op=OP.mult
            )

            # ---- head: all 33 chunks, 2 PE col groups. wl0 has been on-chip
            # since ~40us. Group 0 (even kc + 32) finishes first so its SBUF
            # copy overlaps group 1's matmuls.
            pf_ev = phead.tile([P, 512], F32, tag="pf_ev")
            pf_od = phead.tile([P, 512], F32, tag="pf_od")
            head_started = set()

            def head_mms(kcs, stop_set=()):
                for kc in kcs:
                    g = kc % 2
                    pf = pf_ev if g == 0 else pf_od
                    nc.tensor.matmul(
                        out=pf[32 * g : 32 * g + O, :],
                        lhsT=catFb[:, kc, :],
                        rhs=wl0_sb[:, WL0_POS[kc], :],
                        start=(g not in head_started),
                        stop=(kc in stop_set),
                        skip_group_check=True,
                        tile_position=(0, 32 * g),
                    )
                    head_started.add(g)

            head_mms([kc for kc in range(8, 32)])
            # odd group finishes first; its SBUF copy (scalar) overlaps the
            # even group's remaining matmuls
            head_mms([1, 3, 5, 7], stop_set={7})
            c0 = tmp.tile([O, 512], F32, tag="hc0")
            nc.scalar.copy(out=c0, in_=pf_od[32 : 32 + O, :])
            head_mms([0, 2, 4, 6, 32], stop_set={32})
            # s has a 513th column preset to 1.0; wlrep's 513th column is
            # bl, so the fused relu*wl+reduce also adds the final bias.
            s = tmp.tile([O, 513], F32, tag="hs")
            nc.vector.memset(s[:, 512:513], 1.0)
            nc.vector.tensor_tensor(
                out=s[:, 0:512], in0=pf_ev[0:O, :], in1=c0, op=OP.add
            )
            hw = tmp.tile([O, 513], F32, tag="hw")
            osum = tmp.tile([O, 1], F32, tag="osum")
            nc.vector.scalar_tensor_tensor(
                out=hw, in0=s, scalar=0.0, in1=wlr_sb,
                op0=OP.max, op1=OP.mult, accum_out=osum,
            )
            nc.sync.dma_start(out=out_d[:], in_=osum)

            if debug:
                for name, t, dt in (
                    ("x_all", x_all, F32),
                    ("eTb", eTb, BF16),
                    ("w1", w1x2, F32),
                    ("cat2", cat2, F32),
                    ("rab", rab, F32),
                    ("catFb", catFb, BF16),
                    ("hs", s, F32),
                ):
                    d = nc.dram_tensor(
                        "dbg_" + name, list(t.shape), dt, kind="ExternalOutput"
                    )
                    nc.sync.dma_start(out=d[:], in_=t)

    _split_excess_waits(nc)
    return nc


_NC = None


def _get_nc():
    global _NC
    if _NC is None:
        _NC = _build_nc()
    return _NC


def _prep_inputs(hidden, idx, Wp, bp, Wa0, ba0, Wa, ba, Wl0, bl0, Wl, bl):
    hidden = np.asarray(hidden, dtype=np.float32)
    idx = np.asarray(idx).astype(np.int64)

    f32 = lambda a: np.ascontiguousarray(np.asarray(a, dtype=np.float32))
    bf = lambda a: np.ascontiguousarray(
        np.asarray(a, dtype=np.float32).astype(NPBF16)
    )
    bp, ba0, ba, bl0, bl = f32(bp), f32(ba0), f32(ba), f32(bl0), f32(bl)
    Wl = f32(Wl)

    # [B, O, P, T, E]: partition-major so each DMA run is T*E/2=4KB
    hid8 = np.ascontiguousarray(
        hidden.astype(NPFP8).reshape(B, O, T, P, E).transpose(0, 1, 3, 2, 4)
    )
    wp_t = np.ascontiguousarray((np.asarray(Wp, np.float32).reshape(8, P, 1024).transpose(1, 0, 2) * 128.0).astype(NPFP8))
    wa0_t = np.ascontiguousarray((np.asarray(Wa0, np.float32).reshape(8, P, 512).transpose(1, 0, 2) * 128.0).astype(NPFP8))
    wa_t = np.ascontiguousarray((np.asarray(Wa, np.float32).reshape(4, P, 512).transpose(1, 0, 2) * 128.0).astype(NPFP8))
    wl0_chunks = np.asarray(Wl0, np.float32).reshape(32, P, 512)
    wl0_t = np.zeros((P, 33, 512), dtype=np.float32)
    for pos, kc in enumerate(WL0_ORDER):
        if kc < 32:
            wl0_t[:, pos, :] = wl0_chunks[kc]
        else:
            wl0_t[0, pos, :] = bl0
    wl0_t = bf(wl0_t)

    biasx = np.zeros((1, 16, P), dtype=np.float32)
    biasx[0, 0:8] = bp.reshape(8, P)
    biasx[0, 8:12] = 128.0 * ba0.reshape(4, P)
    biasx[0, 12:16] = 128.0 * ba.reshape(4, P)
    biasx = bf(biasx)

    wlrep = np.zeros((O, 513), dtype=np.float32)
    wlrep[:, 0:512] = Wl[:, 0]
    wlrep[:, 512] = bl[0]
    wlrep = np.ascontiguousarray(wlrep)

    in_maps = []
    for b in range(B):
        m = np.zeros((L, NK), dtype=np.float32)
        cntinv = np.zeros((NK, 1), dtype=np.float32)
        ib = idx[b]
        starts = [1] + [int(ib[k]) for k in range(9)]
        ends = [int(ib[k]) for k in range(10)]
        segs = [(starts[k], ends[k]) for k in range(10)]
        segs.append((int(ib[9]), int(ib[10])))
        segs.append((int(ib[10]), int(ib[11])))
        segs.append((1, int(ib[9])))
        for k, (s, e) in enumerate(segs):
            m[s:e, k] = 1.0
            cntinv[k, 0] = 1.0 / ((e - s) * 128.0)
        maskt = np.ascontiguousarray(
            m.reshape(T, P, NK).transpose(1, 0, 2).astype(NPFP8)
        )

        in_maps.append(
            dict(
                hidden=np.ascontiguousarray(hid8[b]),
                maskt=maskt,
                cntinv=cntinv,
                wp=wp_t,
                wa0=wa0_t,
                wa=wa_t,
                wl0=wl0_t,
                biasx=biasx,
                wlrep=wlrep,
            )
        )
    return in_maps


def _run(in_maps, **kwargs):
    return run_bass_kernel_spmd(_get_nc(), in_maps, core_ids=list(range(B)), **kwargs)


def kernel(**inputs):
    in_maps = _prep_inputs(**inputs)
    res = _run(in_maps)
    return np.stack([r["out"].reshape(O, 1) for r in res.results])


def _install_ntff_hook():
    """The RL container's antenv lacks axon_hooks, so boot() skipped NTFF
    hook registration. Recreate the module and register the ctypes hook."""
    import sys
    import types

    name = "antenv.axon_hooks"
    if name not in sys.modules:
        try:
            __import__(name)
        except ImportError:
            mod = types.ModuleType(name)
            mod._hook = None
            mod.set_axon_ntff_profile_hook = lambda h: setattr(mod, "_hook", h)
            mod.get_axon_ntff_profile_hook = lambda: mod._hook
            sys.modules[name] = mod
            import antenv

            antenv.axon_hooks = mod
    import antenv.axon_hooks as ah

    if ah.get_axon_ntff_profile_hook() is None:
        from trn_agent_boot.trn_boot import _ntff_profile_via_ctypes

        ah.set_axon_ntff_profile_hook(
            _ntff_profile_via_ctypes("/opt/axon/libaxon_pjrt.so")
        )

    import concourse.bass_utils as bu

    bu.upload_artifacts = lambda tmpdir: tmpdir


def benchmark(trace_cores=None, **inputs):
    """Run with NTFF tracing; returns (output, BassKernelResults)."""
    _install_ntff_hook()
    in_maps = _prep_inputs(**inputs)
    res = _run(in_maps, trace=True, trace_cores=trace_cores)
    out = np.stack([r["out"].reshape(O, 1) for r in res.results])
    return out, res


# revision 55
# speedup vs baseline: 1.0409x; 1.0409x over previous
"""Trainium2 Bass kernel for nn_Beta_LR_41308995453190.

Network (per (b, o) pair):
  - 13 segment means over the L axis of hidden[b, o] (ragged boundaries
    from idx[b]): 10 context segments, question, option, whole-context.
  - beta-param projection e = 1 + relu(x @ Wp + bp), split a/b.
  - three attention pools (intersection over segments, renew over
    (segment, intersection) pairs, union over inverted renewed params).
  - classify head: concat 8 beta embeddings -> relu(@Wl0 + bl0) -> @Wl + bl.

Sharding: data-parallel over the batch dim B=8 (one batch per NeuronCore),
weights replicated.

Design (v2 — rebuilt around the trace of the v1 kernel):
  - hidden travels in fp8 e3m4 (4.2 MB/core instead of 8.4 bf16; measured
    end-to-end rel-err 2.9e-3 vs the 2e-2 gate). All weights bf16.
  - Segment sums are 0/1-mask matmuls (mask stationary, hidden streaming).
    The two E-halves run CONCURRENTLY in separate PE column groups
    (tile_position col 0 / 32, derived from the PSUM slice base partition).
  - The beta-network layers run in CLASSIC orientation: weight chunks
    [128, 128] stationary, feature-major activations [128, cols] streaming.
    Layer outputs land feature-major in PSUM, so the bias/relu DVE op is
    128-partition-parallel and NO transposes are needed between layers
    (v1 spent ~10us of PE time on 40 transposes + PSUM round trips).
  - Softmaxes skip the max-subtraction (logits are ~N(0, 0.25)); the
    intersection's exp/weighted sums are reused by the renew stage.
  - Classify head: catF chunks stationary [128, 4], wl0 streams 512 wide,
    accumulated in 4 PE column groups concurrently; bl0 is folded in as a
    33rd contraction chunk (one-hot stationary, bl0 in wl0 row 0). The 24
    chunks that only need the projection run inside the softmax bubbles;
    epilogue relu*Wl+reduce is one fused scalar_tensor_tensor op.
  - DMA: hidden kicks on the Sync HWDGE queue, weights on the Scalar
    queue (two engines issue descriptors concurrently; each descriptor
    fans out to one of 16 HW DMA engines). wl0 (4.2 MB) is ordered last
    — the head only needs it ~25us in.
"""

import numpy as np
import ml_dtypes

try:
    import concourse.bass as bass
except ImportError:
    import sys

    sys.path.insert(0, "/opt/trn_rl_repo")
    import concourse.bass as bass

import concourse.tile as tile
from concourse import mybir
from concourse.bass_utils import run_bass_kernel_spmd
from concourse.masks import make_identity

F32 = mybir.dt.float32
BF16 = mybir.dt.bfloat16
FP8 = mybir.dt.float8e3  # e3m4
NPBF16 = ml_dtypes.bfloat16
NPFP8 = ml_dtypes.float8_e3m4
AX = mybir.AxisListType.X
OP = mybir.AluOpType
AF = mybir.ActivationFunctionType

B, O, L, E = 8, 4, 1024, 1024
BETA = 512
NSEG = 12
NK = 13  # 10 ctx + q + o + allc
NC10 = 10  # pooled ctx segments
P = 128
T = L // P  # 8 L-tiles per option
NCOL = O * NK  # 52

# wl0 host chunk order: natural (the head runs at the very end, after wl0
# has fully arrived; issuing it earlier would stall the in-order PE queue).
WL0_ORDER = list(range(33))
WL0_POS = {kc: i for i, kc in enumerate(WL0_ORDER)}


def _patch_minimal_drain():
    """One-shot NEFF: skip the semaphore-clear + second all-engine barrier of
    the TileContext epilogue (they only matter when the program loops).
    Worth ~6us of measured exec time (the ~150 per-engine semaphore clears
    serialize at 20-50ns each)."""
    from concourse.vector_clock import ScopedClock

    def _drain_and_barrier(self, tick_clock, wait_clock):
        drain_inst = self.nc.sync.drain()
        wait_clock.add_sem_waits(
            drain_inst.ins, ScopedClock({None: tick_clock.global_clock})
        )
        self.nc.all_engine_barrier()
        assert self.sems is not None
        popped = self.nc._tile_sem_poison_stack.pop()
        assert popped is self._sem_poison
    tile.TileContext._drain_and_barrier = _drain_and_barrier


_patch_minimal_drain()


def _split_excess_waits(nc, max_waits=1):
    """This neuronxcc walrus build rejects more than one sem wait per TPB
    instruction ("Too many sync wait commands"). Hoist excess waits onto
    drain instructions inserted immediately before the offending instruction
    on the same engine."""
    scratch_bb = nc.cur_bb.bb
    for f in nc.m.functions:
        for bb in f.blocks:
            new_list = []
            for ins in bb.instructions:
                si = ins.sync_info
                waits = list(si.on_wait) if si and si.on_wait else []
                if len(waits) > max_waits:
                    for w in waits[: len(waits) - max_waits]:
                        carrier = nc.engines[ins.engine].nop(nofuse=True).ins
                        scratch_bb.instructions.remove(carrier)
                        carrier.sync_info = mybir.SyncInfo(
                            on_wait=[w], on_update=[]
                        )
                        new_list.append(carrier)
                    si.on_wait = waits[len(waits) - max_waits :]
                new_list.append(ins)
            bb.instructions[:] = new_list


def _build_nc(debug=False):
    nc = bass.Bass("TRN2", target_bir_lowering=False)

    hid_d = nc.dram_tensor("hidden", [O, P, T, E], FP8, kind="ExternalInput")
    mask_d = nc.dram_tensor("maskt", [P, T, NK], FP8, kind="ExternalInput")
    cnt_d = nc.dram_tensor("cntinv", [NK, 1], F32, kind="ExternalInput")
    wp_d = nc.dram_tensor("wp", [P, 8, 1024], FP8, kind="ExternalInput")
    wa0_d = nc.dram_tensor("wa0", [P, 8, 512], FP8, kind="ExternalInput")
    wa_d = nc.dram_tensor("wa", [P, 4, 512], FP8, kind="ExternalInput")
    wl0_d = nc.dram_tensor("wl0", [P, 33, 512], BF16, kind="ExternalInput")
    biasx_d = nc.dram_tensor("biasx", [1, 16, P], BF16, kind="ExternalInput")
    wlr_d = nc.dram_tensor("wlrep", [O, 513], F32, kind="ExternalInput")
    out_d = nc.dram_tensor("out", [O, 1], F32, kind="ExternalOutput")

    with tile.TileContext(nc) as tc:
        with (
            tc.tile_pool(name="const", bufs=1) as const,
            tc.tile_pool(name="hidp2", bufs=2) as hidp2,
            tc.tile_pool(name="act", bufs=1) as act,
            tc.tile_pool(name="tmp", bufs=2) as tmp,
            tc.tile_pool(name="pseg", bufs=2, space="PSUM") as pseg,
            tc.tile_pool(name="pt", bufs=2, space="PSUM") as pt,
            tc.tile_pool(name="pmm", bufs=2, space="PSUM") as pmm,
            tc.tile_pool(name="phead", bufs=1, space="PSUM") as phead,
        ):
            # ---- DMA kicks. Two HWDGE queues (Sync, Scalar) issue
            # descriptors concurrently; each descriptor fans out to one of
            # 16 HW DMA engines (~24.5 GB/s per engine, ~358 GB/s aggregate,
            # FIFO per engine). Hidden options are interleaved across both
            # queues so all 16 hidden chunks land on distinct engines first;
            # weights follow (wp/wa0 split by output block so the consuming
            # layers pipeline); wl0 is last (head needs it at ~40us).
            mask_sb = const.tile([P, T, NK], FP8)
            nc.sync.dma_start(out=mask_sb, in_=mask_d[:])
            # touch the activation table now so ACT_TABLE_LOAD (~1.3us)
            # overlaps the initial DMA instead of stalling the first use
            actwarm = const.tile([1, 1], F32)
            nc.vector.memset(actwarm, 1.0)
            nc.scalar.activation(out=actwarm, in_=actwarm, func=AF.Exp)
            cnt_sb = const.tile([NK, 1], F32)
            nc.scalar.dma_start(out=cnt_sb, in_=cnt_d[:])
            wlr_sb = const.tile([O, 513], F32)
            nc.scalar.dma_start(out=wlr_sb, in_=wlr_d[:])
            # bias-as-matmul: every layer chain gets one extra contraction
            # chunk (ones-row moving x bias-row stationary, both bf16) so
            # the PSUM already carries the bias and the downstream relu/exp
            # is a single wide op with no per-m-chunk bias columns.
            biasx_sb = const.tile([P, 16, P], BF16)
            nc.vector.memset(biasx_sb, 0.0)
            nc.scalar.dma_start(out=biasx_sb[0:1, :, :], in_=biasx_d[:])
            ones52 = const.tile([P, O, NK], BF16)
            nc.vector.memset(ones52, 0.0)
            nc.vector.memset(ones52[0:1, :, :], 1.0)

            # Few big descriptors in strict need-order: each descriptor
            # sustains ~170 GB/s on its HW engine, so ~2 in flight saturate
            # the ~358 GB/s aggregate. One option = 2 descriptors (one per
            # HWDGE queue); weights follow; wl0 last (head needs it ~45us).
            # A queue round-robins across ALL its pending descriptors, so
            # queue position gives no priority and concurrent transfers
            # steal bandwidth from hidden. Build a strict transfer pipeline
            # o0 -> o1 -> o2 -> o3 -> wp -> wa0/wa -> wl0 by gating each
            # kick on the previous tensor's LAST bytes via 1-element dummy
            # copies (WAW dep -> sem wait on the kick). Sequential options
            # also keep the PE continuously fed (HAM stays warm).
            hid_r = hid_d
            htiles = [
                hidp2.tile([P, T, E], FP8, tag="htile", name=f"htile{o}")
                for o in range(O)
            ]

            def gate(dst_tile, srcs):
                for j, s in enumerate(srcs):
                    nc.gpsimd.tensor_copy(
                        out=dst_tile[0:1, 0, j : j + 1], in_=s
                    )

            def hid_last(o):
                # any element works: the DMA semaphore bumps once per
                # descriptor, so the dep resolves at descriptor completion
                return [htiles[o][0:1, 3, 0:1], htiles[o][0:1, 7, 0:1]]

            for o in range(O):
                nc.sync.dma_start(
                    out=htiles[o][:, 0:4, :], in_=hid_r[o][:, 0:4, :]
                )
                nc.scalar.dma_start(
                    out=htiles[o][:, 4:8, :], in_=hid_r[o][:, 4:8, :]
                )
            wp_sb = const.tile([P, 8, 1024], FP8)
            gate(wp_sb, hid_last(1))
            nc.sync.dma_start(out=wp_sb[:, 0:4, :], in_=wp_d[:, 0:4, :])
            nc.scalar.dma_start(out=wp_sb[:, 4:8, :], in_=wp_d[:, 4:8, :])
            wp_last = [wp_sb[0:1, 3, 0:1], wp_sb[0:1, 7, 0:1]]

            wa0_sb = const.tile([P, 8, 512], FP8)
            gate(wa0_sb, hid_last(2))
            wa_sb = const.tile([P, 4, 512], FP8)
            gate(wa_sb, hid_last(2))
            nc.sync.dma_start(out=wa0_sb, in_=wa0_d[:])
            nc.scalar.dma_start(out=wa_sb, in_=wa_d[:])
            wl0_sb = const.tile([P, 33, 512], BF16)
            gate(wl0_sb, hid_last(3))
            for j, sl in enumerate(
                (slice(0, 8), slice(8, 16), slice(16, 25), slice(25, 33))
            ):
                eng = nc.sync if j % 2 == 0 else nc.scalar
                eng.dma_start(out=wl0_sb[:, sl, :], in_=wl0_d[:, sl, :])

            ident = const.tile([P, P], BF16)
            make_identity(nc, ident)

            # ---- phase A: segment sums. ps[k, e] = sum over rows of seg k.
            # The two E-halves run concurrently in PE col groups 0 and 1
            # (group from the PSUM slice base partition: 0 -> cols 0-31,
    # 32 -> cols 32-63).
            x_all = act.tile([P, E], BF16, tag="x_all")
            for o in range(O):
                htile = htiles[o]
                ps = pseg.tile([P, 512], F32, tag="ps_seg")
                for t in range(T):
                    nc.tensor.matmul(
                        out=ps[0:NK, :],
                        lhsT=mask_sb[:, t, :],
                        rhs=htile[:, t, 0:512],
                        start=(t == 0),
                        stop=(t == T - 1),
                        skip_group_check=True,
                    )
                    nc.tensor.matmul(
                        out=ps[32 : 32 + NK, :],
                        lhsT=mask_sb[:, t, :],
                        rhs=htile[:, t, 512:1024],
                        start=(t == 0),
                        stop=(t == T - 1),
                        skip_group_check=True,
                    )
                nc.vector.tensor_scalar_mul(
                    out=x_all[o * 32 : o * 32 + NK, 0:512],
                    in0=ps[0:NK, :],
                    scalar1=cnt_sb[:, :],
                )
                nc.vector.tensor_scalar_mul(
                    out=x_all[o * 32 : o * 32 + NK, 512:1024],
                    in0=ps[32 : 32 + NK, :],
                    scalar1=cnt_sb[:, :],
                )

            # ---- xT: transpose to feature-major [128, 8, O, NK] bf16
            xTb = act.tile([P, 8, O, 32], BF16)
            for c in range(8):
                ptile = pt.tile([P, P], BF16, tag="pt")
                nc.tensor.transpose(
                    out=ptile,
                    in_=x_all[:, c * P : (c + 1) * P],
                    identity=ident[:, :],
                )
                nc.scalar.copy(
                    out=xTb[:, c, :, 0:NK],
                    in_=ptile.rearrange("p (o k) -> p o k", k=32)[:, :, 0:NK],
                )

            # ---- e = max(x @ Wp + (bp+1), 1), classic orientation:
            # Wp [128,128] chunks stationary, xTb streams 52 cols. k-outer
            # so the matmuls start as each wp k-chunk lands from DMA.
            # NOTE: chains must run one-at-a-time (m-outer): a matmul with
            # start=True clears its full partition range in the target bank,
            # so interleaving same-partition accumulation chains loses the
            # earlier chains' first terms (verified on HW).
            pe_ps = pmm.tile([P, 8, O, NK], F32, tag="pml")
            for m in range(8):
                nc.tensor.matmul(
                    out=pe_ps[:, m, :, :], lhsT=biasx_sb[:, m, :],
                    rhs=ones52, start=True, stop=False,
                )
                for k in range(8):
                    nc.tensor.matmul(
                        out=pe_ps[:, m, :, :],
                        lhsT=wp_sb[:, k, m * P : (m + 1) * P],
                        rhs=xTb[:, k, :, 0:NK],
                        start=False,
                        stop=(k == 7),
                    )
            # erf = relu(z + bp) on the scalar engine (pipelines behind the
            # m-chunk psums); e = erf + 1 is ONE wide vector op. Pool sums
            # use erf via fused (erf + 1) * w scalar_tensor_tensor ops.
            erf = act.tile([P, 8, O, NK], F32)
            nc.scalar.activation(out=erf, in_=pe_ps, func=AF.Relu)
            eTb = act.tile([P, 8, O, NK], BF16)
            nc.vector.tensor_scalar_add(out=eTb, in0=erf, scalar1=1.0)

            # catF chunks 8..31 (a_ac,b_ac,a_o,b_o,a_q,b_q) only need e.
            catFb = act.tile([P, 33, O], BF16)
            for j, (half, k) in enumerate(
                ((0, 12), (1, 12), (0, 11), (1, 11), (0, 10), (1, 10))
            ):
                nc.gpsimd.tensor_copy(
                    out=catFb[:, 8 + j * 4 : 12 + j * 4, :],
                    in_=eTb[:, half * 4 : half * 4 + 4, :, k],
                )
            # bl0 folding chunk: one-hot stationary row (partition 0 = 1).
            nc.gpsimd.memset(catFb[:, 32, :], 0.0)
            nc.gpsimd.memset(catFb[0:1, 32, :], 1.0)

            def layer(name, w_sb, nk_chunks, rhs_fn, nm, out_free, brow, ones):
                psl = pmm.tile(
                    [P, nm] + out_free, F32, tag="pml", name=f"ps_{name}"
                )
                for m in range(nm):
                    nc.tensor.matmul(
                        out=psl[:, m], lhsT=biasx_sb[:, brow + m, :],
                        rhs=ones, start=True, stop=False,
                    )
                    for k in range(nk_chunks):
                        nc.tensor.matmul(
                            out=psl[:, m],
                            lhsT=w_sb[:, k, m * P : (m + 1) * P],
                            rhs=rhs_fn(k),
                            start=False,
                            stop=(k == nk_chunks - 1),
                        )
                return psl

            # ---- pool 1 (intersection) over the 10 ctx segments.
            # Bias+relu and bias+exp run as single scalar-engine activation
            # ops straight out of PSUM (out = func(in*scale + bias)).
            ones40 = ones52[:, :, 0:NC10]
            ph1 = layer("h1", wa0_sb, 8, lambda k: eTb[:, k, :, 0:NC10], 4,
                        [O, NC10], 8, ones40)
            h1b = act.tile([P, 4, O, NC10], BF16)
            nc.vector.tensor_scalar(
                out=h1b, in0=ph1, scalar1=1.0 / 128.0, scalar2=0.0,
                op0=OP.mult, op1=OP.max,
            )
            pl1 = layer("l1", wa_sb, 4, lambda k: h1b[:, k], 4, [O, NC10],
                        12, ones40)

            # softmax over segments, no max-subtraction (|l1| < ~1.5).
            # Shared factors (w1, r1, ...) are duplicated across the a/b
            # chunk halves by gpsimd copies so each chain step is ONE wide
            # vector op instead of two serialized halves.
            w1x2 = act.tile([P, 8, O, NC10], F32)
            w1 = w1x2[:, 0:4]
            nc.scalar.activation(
                out=w1, in_=pl1, func=AF.Exp, scale=1.0 / 128.0
            )
            nc.gpsimd.tensor_copy(out=w1x2[:, 4:8], in_=w1)
            s1 = act.tile([P, 4, O], F32)
            nc.vector.reduce_sum(s1, w1, axis=AX)
            r1x2 = act.tile([P, 8, O], F32)
            nc.vector.reciprocal(out=r1x2[:, 0:4, :], in_=s1)
            nc.gpsimd.tensor_copy(out=r1x2[:, 4:8, :], in_=r1x2[:, 0:4, :])
            wab_t = act.tile([P, 8, O, NC10], F32)
            nc.vector.scalar_tensor_tensor(
                out=wab_t, in0=erf[:, :, :, 0:NC10], scalar=1.0, in1=w1x2,
                op0=OP.add, op1=OP.mult,
            )
            sumab = tmp.tile([P, 8, O], F32, tag="sumab")
            nc.vector.reduce_sum(sumab, wab_t, axis=AX)
            cat2 = act.tile([P, 8, O], F32)
            nc.vector.tensor_tensor(out=cat2, in0=sumab, in1=r1x2, op=OP.mult)
            cat2b = act.tile([P, 8, O], BF16)
            nc.vector.tensor_copy(out=cat2b, in_=cat2)

            # ---- renew: h2/l2 on the intersection [O] columns
            ones4 = ones52[:, :, 0:1]
            ph2 = layer("h2", wa0_sb, 8, lambda k: cat2b[:, k, :], 4, [O],
                        8, ones4)
            h2b = act.tile([P, 4, O], BF16)
            nc.vector.tensor_scalar(
                out=h2b, in0=ph2, scalar1=1.0 / 128.0, scalar2=0.0,
                op0=OP.mult, op1=OP.max,
            )
            pl2 = layer("l2", wa_sb, 4, lambda k: h2b[:, k], 4, [O], 12, ones4)

            # pair softmax([l1, l2]) -> na/nb, store reciprocals.
            # e1 = exp(l1) = w1 (reused), e2 = exp(l2).
            def warm_fill(n):
                # independent matmuls into a scratch bank: keep the PE HAM
                # at K=8/8 through the softmax bubbles (idle >3.4us would
                # re-throttle the clock to 1.2 GHz and slow the next burst)
                pscr = pt.tile([P, 512], F32, tag="pt", name="pscr")
                for i in range(n):
                    nc.tensor.matmul(
                        out=pscr[0:O, :], lhsT=catFb[:, 8, :],
                        rhs=wl0_sb[:, 0, :], start=True, stop=True,
                        skip_group_check=True,
                    )

            e2x2 = act.tile([P, 8, O], F32)
            e2 = e2x2[:, 0:4, :]
            nc.scalar.activation(out=e2, in_=pl2, func=AF.Exp, scale=1.0 / 128.0)
            nc.gpsimd.tensor_copy(out=e2x2[:, 4:8, :], in_=e2)
            s12 = tmp.tile([P, 4, O, NC10], F32, tag="s12")
            nc.vector.tensor_tensor(
                out=s12, in0=w1, in1=e2.broadcast_to([P, 4, O, NC10]), op=OP.add
            )
            s12x2 = act.tile([P, 8, O, NC10], F32)
            nc.gpsimd.tensor_copy(out=s12x2[:, 0:4], in_=s12)
            nc.gpsimd.tensor_copy(out=s12x2[:, 4:8], in_=s12)
            t2ab = tmp.tile([P, 8, O], F32, tag="t2ab")
            nc.vector.tensor_tensor(out=t2ab, in0=e2x2, in1=cat2, op=OP.mult)
            t3ab = tmp.tile([P, 8, O, NC10], F32, tag="t3ab")
            nc.vector.tensor_tensor(
                out=t3ab, in0=wab_t,
                in1=t2ab.broadcast_to([P, 8, O, NC10]), op=OP.add,
            )
            # 1/na = s12 * (1/(w1*a + e2*ia)): one wide reciprocal + mult
            # (DVE/Pool reject op=divide; two-recip version costs 1.2us more)
            rt3 = tmp.tile([P, 8, O, NC10], F32, tag="rt3")
            nc.vector.reciprocal(out=rt3, in_=t3ab)
            rab = act.tile([P, 8, O, NC10], F32)
            nc.vector.tensor_tensor(out=rab, in0=s12x2, in1=rt3, op=OP.mult)
            rabb = act.tile([P, 8, O, NC10], BF16)
            nc.vector.tensor_copy(out=rabb, in_=rab)

            # ---- union pool over segments of [1/na; 1/nb]
            ph3 = layer("h3", wa0_sb, 8, lambda k: rabb[:, k], 4,
                        [O, NC10], 8, ones40)
            h3b = act.tile([P, 4, O, NC10], BF16)
            nc.vector.tensor_scalar(
                out=h3b, in0=ph3, scalar1=1.0 / 128.0, scalar2=0.0,
                op0=OP.mult, op1=OP.max,
            )
            pl3 = layer("l3", wa_sb, 4, lambda k: h3b[:, k], 4, [O, NC10],
                        12, ones40)

            w3x2 = act.tile([P, 8, O, NC10], F32)
            w3 = w3x2[:, 0:4]
            nc.scalar.activation(out=w3, in_=pl3, func=AF.Exp, scale=1.0 / 128.0)
            nc.gpsimd.tensor_copy(out=w3x2[:, 4:8], in_=w3)
            s3x2 = act.tile([P, 8, O], F32)
            nc.vector.reduce_sum(s3x2[:, 0:4, :], w3, axis=AX)
            nc.gpsimd.tensor_copy(out=s3x2[:, 4:8, :], in_=s3x2[:, 0:4, :])
            # ua = s3 / sum(w3 * ra) ; ub likewise, one wide op per step
            tuab = tmp.tile([P, 8, O, NC10], F32, tag="tuab")
            nc.vector.tensor_tensor(out=tuab, in0=w3x2, in1=rab, op=OP.mult)
            suab = tmp.tile([P, 8, O], F32, tag="suab")
            nc.vector.reduce_sum(suab, tuab, axis=AX)
            invab = tmp.tile([P, 8, O], F32, tag="invab")
            nc.vector.reciprocal(out=invab, in_=suab)
            nc.vector.tensor_tensor(
                out=catFb[:, 0:8, :], in0=s3x2, in1=invab, op=OP.mult
            )

            # ---- head: all 33 chunks, 2 PE col groups. wl0 has been on-chip
            # since ~40us. Group 0 (even kc + 32) finishes first so its SBUF
            # copy overlaps group 1's matmuls.
            pf_ev = phead.tile([P, 512], F32, tag="pf_ev")
            pf_od = phead.tile([P, 512], F32, tag="pf_od")
            head_started = set()

            def head_mms(kcs, stop_set=()):
                for kc in kcs:
                    g = kc % 2
                    pf = pf_ev if g == 0 else pf_od
                    nc.tensor.matmul(
                        out=pf[32 * g : 32 * g + O, :],
                        lhsT=catFb[:, kc, :],
                        rhs=wl0_sb[:, WL0_POS[kc], :],
                        start=(g not in head_started),
                        stop=(kc in stop_set),
                        skip_group_check=True,
                        tile_position=(0, 32 * g),
                    )
                    head_started.add(g)

            head_mms([kc for kc in range(8, 32)])
            # odd group finishes first; its SBUF copy (scalar) overlaps the
            # even group's remaining matmuls
            head_mms([1, 3, 5, 7], stop_set={7})
            c0 = tmp.tile([O, 512], F32, tag="hc0")
            nc.scalar.copy(out=c0, in_=pf_od[32 : 32 + O, :])
            head_mms([0, 2, 4, 6, 32], stop_set={32})
            # s has a 513th column preset to 1.0; wlrep's 513th column is
            # bl, so the fused relu*wl+reduce also adds the final bias.
            s = tmp.tile([O, 513], F32, tag="hs")
            nc.vector.memset(s[:, 512:513], 1.0)
            nc.vector.tensor_tensor(
                out=s[:, 0:512], in0=pf_ev[0:O, :], in1=c0, op=OP.add
            )
            hw = tmp.tile([O, 513], F32, tag="hw")
            osum = tmp.tile([O, 1], F32, tag="osum")
            nc.vector.scalar_tensor_tensor(
                out=hw, in0=s, scalar=0.0, in1=wlr_sb,
                op0=OP.max, op1=OP.mult, accum_out=osum,
            )
            nc.sync.dma_start(out=out_d[:], in_=osum)

            if debug:
                for name, t, dt in (
                    ("x_all", x_all, F32),
                    ("eTb", eTb, BF16),
                    ("w1", w1x2, F32),
                    ("cat2", cat2, F32),
                    ("rab", rab, F32),
                    ("catFb", catFb, BF16),
                    ("hs", s, F32),
                ):
                    d = nc.dram_tensor(
                        "dbg_" + name, list(t.shape), dt, kind="ExternalOutput"
                    )
                    nc.sync.dma_start(out=d[:], in_=t)

    _split_excess_waits(nc)
    return nc


_NC = None


def _get_nc():
    global _NC
    if _NC is None:
        _NC = _build_nc()
    return _NC


def _prep_inputs(hidden, idx, Wp, bp, Wa0, ba0, Wa, ba, Wl0, bl0, Wl, bl):
    hidden = np.asarray(hidden, dtype=np.float32)
    idx = np.asarray(idx).astype(np.int64)

    f32 = lambda a: np.ascontiguousarray(np.asarray(a, dtype=np.float32))
    bf = lambda a: np.ascontiguousarray(
        np.asarray(a, dtype=np.float32).astype(NPBF16)
    )
    bp, ba0, ba, bl0, bl = f32(bp), f32(ba0), f32(ba), f32(bl0), f32(bl)
    Wl = f32(Wl)

    # [B, O, P, T, E]: partition-major so each DMA run is T*E/2=4KB
    hid8 = np.ascontiguousarray(
        hidden.astype(NPFP8).reshape(B, O, T, P, E).transpose(0, 1, 3, 2, 4)
    )
    wp_t = np.ascontiguousarray((np.asarray(Wp, np.float32).reshape(8, P, 1024).transpose(1, 0, 2) * 128.0).astype(NPFP8))
    wa0_t = np.ascontiguousarray((np.asarray(Wa0, np.float32).reshape(8, P, 512).transpose(1, 0, 2) * 128.0).astype(NPFP8))
    wa_t = np.ascontiguousarray((np.asarray(Wa, np.float32).reshape(4, P, 512).transpose(1, 0, 2) * 128.0).astype(NPFP8))
    wl0_chunks = np.asarray(Wl0, np.float32).reshape(32, P, 512)
    wl0_t = np.zeros((P, 33, 512), dtype=np.float32)
    for pos, kc in enumerate(WL0_ORDER):
        if kc < 32:
            wl0_t[:, pos, :] = wl0_chunks[kc]
        else:
            wl0_t[0, pos, :] = bl0
    wl0_t = bf(wl0_t)

    biasx = np.zeros((1, 16, P), dtype=np.float32)
    biasx[0, 0:8] = bp.reshape(8, P)
    biasx[0, 8:12] = 128.0 * ba0.reshape(4, P)
    biasx[0, 12:16] = 128.0 * ba.reshape(4, P)
    biasx = bf(biasx)

    wlrep = np.zeros((O, 513), dtype=np.float32)
    wlrep[:, 0:512] = Wl[:, 0]
    wlrep[:, 512] = bl[0]
    wlrep = np.ascontiguousarray(wlrep)

    in_maps = []
    for b in range(B):
        m = np.zeros((L, NK), dtype=np.float32)
        cntinv = np.zeros((NK, 1), dtype=np.float32)
        ib = idx[b]
        starts = [1] + [int(ib[k]) for k in range(9)]
        ends = [int(ib[k]) for k in range(10)]
        segs = [(starts[k], ends[k]) for k in range(10)]
        segs.append((int(ib[9]), int(ib[10])))
        segs.append((int(ib[10]), int(ib[11])))
        segs.append((1, int(ib[9])))
        for k, (s, e) in enumerate(segs):
            m[s:e, k] = 1.0
            cntinv[k, 0] = 1.0 / ((e - s) * 128.0)
        maskt = np.ascontiguousarray(
            m.reshape(T, P, NK).transpose(1, 0, 2).astype(NPFP8)
        )

        in_maps.append(
            dict(
                hidden=np.ascontiguousarray(hid8[b]),
                maskt=maskt,
                cntinv=cntinv,
                wp=wp_t,
                wa0=wa0_t,
                wa=wa_t,
                wl0=wl0_t,
                biasx=biasx,
                wlrep=wlrep,
            )
        )
    return in_maps


def _run(in_maps, **kwargs):
    return run_bass_kernel_spmd(_get_nc(), in_maps, core_ids=list(range(B)), **kwargs)


def kernel(**inputs):
    in_maps = _prep_inputs(**inputs)
    res = _run(in_maps)
    return np.stack([r["out"].reshape(O, 1) for r in res.results])


def _install_ntff_hook():
    """The RL container's antenv lacks axon_hooks, so boot() skipped NTFF
    hook registration. Recreate the module and register the ctypes hook."""
    import sys
    import types

    name = "antenv.axon_hooks"
    if name not in sys.modules:
        try:
            __import__(name)
        except ImportError:
            mod = types.ModuleType(name)
            mod._hook = None
            mod.set_axon_ntff_profile_hook = lambda h: setattr(mod, "_hook", h)
            mod.get_axon_ntff_profile_hook = lambda: mod._hook
            sys.modules[name] = mod
            import antenv

            antenv.axon_hooks = mod
    import antenv.axon_hooks as ah

    if ah.get_axon_ntff_profile_hook() is None:
        from trn_agent_boot.trn_boot import _ntff_profile_via_ctypes

        ah.set_axon_ntff_profile_hook(
            _ntff_profile_via_ctypes("/opt/axon/libaxon_pjrt.so")
        )

    import concourse.bass_utils as bu

    bu.upload_artifacts = lambda tmpdir: tmpdir


def benchmark(trace_cores=None, **inputs):
    """Run with NTFF tracing; returns (output, BassKernelResults)."""
    _install_ntff_hook()
    in_maps = _prep_inputs(**inputs)
    res = _run(in_maps, trace=True, trace_cores=trace_cores)
    out = np.stack([r["out"].reshape(O, 1) for r in res.results])
    return out, res
